# revision 10
# baseline (speedup 1.0000x reference)
"""Trainium2 Bass kernel for a (buggy-but-well-defined) ConvTranspose2d.

Math (matches the reference exactly):
  out[b, co, i, j] = sum_{ci,kh,kw} ker[ci,co,3-kh,3-kw] * x[b,ci,i+kh-3,j+kw-3]
                     + bias_sum * cnt[i] * cnt[j]          for i,j in [0,66)
  (terms with i+kh-3 or j+kw-3 outside [0,63) are dropped), and out is zero
  elsewhere in the (B,128,126,126) output.

Strategy: data-parallel over batch (2 items / core on 8 cores).  Per core,
the 66 output rows are split into 10 groups (9x7 + 1x3 rows); each group
accumulates its [128, r*66] tile in one PSUM bank via up to 16 shifted
128x128 bf16 matmuls (contraction over ci on the partition dim).  The image
is stored UNPADDED (63x63) in SBUF: every matmul reads exactly the valid
63-wide row segments and writes a row/col-trimmed window of the PSUM tile
(out-of-range taps contribute nothing and are skipped), which cuts PE
streaming work ~9% vs the padded formulation.  start=True zeroes the whole
2KB PSUM zero-region, so partial first-tap footprints are safe.

Groups are processed in sets of {5,4,1} with the tap loop OUTER, so
consecutive matmuls share the stationary weights; a post-legalization pass
then deletes the redundant InstLdweights, which removes the per-matmul
weight-swap pipeline bubble (~45ns) for all but the first matmul of each
tap.  A burst of throwaway warm-up matmuls runs while the input DMA is in
flight so the PE's HAM clock-gate is already at 8/8 when the real stream
starts.  Everything on-chip is bf16 (fp32 PSUM accumulation); the rank-1
bias field and the zero border are applied host-side during assembly.
"""

import ml_dtypes
import numpy as np

import concourse.bacc as bacc
import concourse.mybir as mybir
import concourse.tile as tile
from concourse.bass_utils import run_bass_kernel_spmd

B, CIN, COUT, K, H, W = 16, 128, 128, 4, 64, 64
NCORES = 8
BPC = B // NCORES          # batch items per core
HV = H - 1                 # 63 valid input rows/cols
HO = HV + K - 1            # 66 output rows/cols (nonzero region)
HOUT = (H - 1) * 2         # 126 full output rows/cols
NWT = K * K * COUT         # 2048 weight cols
NXI = HV * HV              # 3969 unpadded-image cols per batch item
NXW = NWT + BPC * NXI      # merged wt+image tensor cols
F32 = mybir.dt.float32
BF16 = mybir.dt.bfloat16
BF16NP = ml_dtypes.bfloat16

GROUPS = [(0, 7), (7, 7), (14, 7), (21, 7), (28, 7),
          (35, 7), (42, 7), (49, 7), (56, 7), (63, 3)]
# Groups per set share each stationary-weight load (taps loop is outer), so
# fewer sets = fewer weight swaps.  Item 0 starts with a small set (less
# input needed before the stream starts); item 1 ends with a small set
# (short drain after the last matmul).  Max set size 7 = PSUM banks minus
# the warm-up bank.
SETS = {0: [(0, 1, 2), (3, 4, 5, 6, 7, 8, 9)],
        1: [(0, 1, 2, 3, 4, 5, 6), (7, 8, 9)]}
NWARM = 8                  # PE warm-up matmuls during the input-DMA head
NWCOL = 500                # their free dim
DEDUPE_LDW = True


def _plan(i0, r):
    """Per-group tap plan: (t, kh, kw, rs, re, ro) with zero-work taps gone."""
    plan = []
    for t in range(K * K):
        kh, kw = divmod(t, K)
        rs = max(0, i0 + kh - 3)
        re = min(HV, i0 + r + kh - 3)
        if re > rs:
            plan.append((t, kh, kw, rs, re, rs + 3 - kh - i0))
    return plan


def _dedupe_ldweights(nc):
    """Drop an InstLdweights whose weights AP matches the previous PE weight
    load with only InstMatmult/sync instructions in between: the array
    already holds those weights, and skipping the reload removes the
    per-matmul weight-swap bubble.  Only sync-free loads are dropped."""
    ndrop = 0
    for blk in nc.main_func.blocks:
        keep, prev_key = [], None
        for inst in blk.instructions:
            if getattr(inst, "engine", None) == mybir.EngineType.PE:
                if isinstance(inst, mybir.InstLdweights):
                    ap = inst.ins[0]
                    key = (str(ap.ap), ap.offset, str(ap.memref),
                           str(ap.dtype))
                    si = inst.sync_info
                    clean = si is None or (not si.on_wait and not si.on_update)
                    if key == prev_key and clean:
                        ndrop += 1
                        continue
                    prev_key = key
                elif not isinstance(inst, (mybir.InstMatmult,
                                           mybir.InstEventSemaphore)):
                    prev_key = None   # unknown PE op: don't reuse across it
            keep.append(inst)
        blk.instructions[:] = keep
    return ndrop


_CACHE = {}


def _build_nc():
    # Bacc (not raw Bass): its finalize() legalizes sync waits — moving
    # excess matmul waits onto LDWEIGHTS and splitting multi-waits onto
    # EventSemaphore instructions — which walrus codegen requires.
    nc = bacc.Bacc(None)
    xw = nc.dram_tensor("xw", [CIN, NXW], BF16, kind="ExternalInput")
    out = nc.dram_tensor("out", [BPC, COUT, HO, HO], BF16,
                         kind="ExternalOutput")

    with tile.TileContext(nc) as tc:
        with (
            tc.tile_pool(name="wpool", bufs=1) as wpool,
            tc.tile_pool(name="wps", bufs=1, space="PSUM") as wps_pool,
            tc.tile_pool(name="xwpool", bufs=1) as xwpool,
            tc.tile_pool(name="acc", bufs=7, space="PSUM") as psum_pool,
            tc.tile_pool(name="opool", bufs=6) as opool,
        ):
            # PE warm-up: dummy matmuls on a zeroed scratch tile keep the PE
            # array busy from right after the engine prologue, so the HAM
            # clock-gate reaches 8/8 before the first real matmul and the
            # input-DMA wait is hidden behind array activity.  They all share
            # one stationary load (deduped below), so the stream is dense.
            scr = wpool.tile([CIN, NWCOL], BF16)
            nc.vector.memzero(scr)
            wps = wps_pool.tile([CIN, NWCOL], F32)
            for _ in range(NWARM):
                nc.tensor.matmul(wps, scr[:, :CIN], scr, start=True, stop=True)

            xwt = xwpool.tile([CIN, NXW], BF16)
            # Input chunks in arrival order: tap-0 weights, then the first
            # set's image rows, then the rest.  The non-urgent chunks start
            # one column early (re-writing a column already covered, same
            # data): the overlap is a WAW dependency that serializes them
            # behind the critical second chunk, so they don't dilute its
            # DMA bandwidth while the PE is waiting on it.
            iw = lambda a, b: (NWT + a * HV, NWT + b * HV)  # item-0 row cols
            chunks = [(0, COUT),                   # tap-0 weights
                      iw(0, 11),                   # rows for set0 taps 0, g0-1
                      (COUT, NWT),                 # remaining weights
                      (iw(0, 11)[1] - 1, iw(0, 22)[1]),   # rest of set-0 rows
                      (iw(0, 22)[1] - 1, NWT + NXI),      # rest of item 0
                      (NWT + NXI - 1, NWT + NXI + 50 * HV),  # item-1 set 0
                      (NWT + NXI + 50 * HV - 1, NXW)]        # item-1 rest
            for a, b in chunks:
                nc.sync.dma_start(xwt[:, a:b], xw[:, a:b])

            xs = xwt[:, NWT:].rearrange("p (b h w) -> p b h w",
                                        b=BPC, h=HV, w=HV)

            for b in range(BPC):
                for st in SETS[b]:
                    plans = {g: _plan(*GROUPS[g]) for g in st}
                    accs = {}
                    for g in st:
                        i0, r = GROUPS[g]
                        acc = psum_pool.tile([COUT, 7 * HO], F32,
                                             tag="acc", name="acc")
                        accs[g] = acc[:, :r * HO].rearrange(
                            "p (r c) -> p r c", r=r, c=HO)
                    for t in range(K * K):
                        lhsT = xwt[:, t * COUT:(t + 1) * COUT]
                        for g in st:
                            steps = [s for s in plans[g] if s[0] == t]
                            if not steps:
                                continue
                            _, kh, kw, rs, re, ro = steps[0]
                            i0, r = GROUPS[g]
                            n = re - rs
                            nc.tensor.matmul(
                                accs[g][:, ro:ro + n, 3 - kw:HO - kw],
                                lhsT, xs[:, b, rs:re, :],
                                start=(t == plans[g][0][0]),
                                stop=(t == plans[g][-1][0]))
                            if t == plans[g][-1][0]:
                                # Close the group right after its last tap so
                                # the PSUM->SBUF cast and out-DMA overlap the
                                # remaining matmuls.  Casts alternate between
                                # DVE and ACT so bunched closes pipeline
                                # 2-wide; out-DMAs ride the Sync queue, idle
                                # after the input loads.
                                otile = opool.tile([COUT, 7 * HO], BF16,
                                                   tag="ot", name="ot")
                                flat = accs[g].rearrange("p r c -> p (r c)")
                                if g % 2:
                                    nc.scalar.activation(
                                        otile[:, :r * HO], flat,
                                        mybir.ActivationFunctionType.Copy)
                                    dma = nc.scalar.dma_start
                                else:
                                    nc.vector.tensor_copy(
                                        otile[:, :r * HO], flat)
                                    dma = nc.sync.dma_start
                                dma(out[b, :, i0:i0 + r, :],
                                    otile[:, :r * HO])
    if DEDUPE_LDW:
        _dedupe_ldweights(nc)
    nc.finalize()
    return nc


def get_nc():
    if "nc" not in _CACHE:
        _CACHE["nc"] = _build_nc()
    return _CACHE["nc"]


def prep_inputs(x, kernel, bias):
    """Host-side prep: per-core input maps (numpy only, negligible cost)."""
    x = np.asarray(x, dtype=np.float32)
    ker = np.asarray(kernel, dtype=np.float32)

    kf = ker[:, :, ::-1, ::-1]                        # [ci, co, kh, kw] flipped
    wt = np.ascontiguousarray(kf.transpose(0, 2, 3, 1)).reshape(
        CIN, NWT).astype(BF16NP)                      # [ci, (kh kw co)]
    xv = x[:, :, :HV, :HV].astype(BF16NP)             # [B, ci, 63, 63]

    in_maps = []
    for c in range(NCORES):
        xwa = np.empty((CIN, NXW), BF16NP)
        xwa[:, :NWT] = wt
        xwa[:, NWT:] = xv[c * BPC:(c + 1) * BPC].transpose(1, 0, 2, 3) \
            .reshape(CIN, BPC * NXI)
        in_maps.append({"xw": xwa})
    return in_maps


def assemble(per_core_outs, bias):
    bias = np.asarray(bias, dtype=np.float32)
    cnt = np.convolve(np.ones(HV, np.float32), np.ones(K, np.float32))
    bfield = np.sum(bias[:COUT], dtype=np.float32) * np.outer(cnt, cnt)

    out = np.zeros((B, COUT, HOUT, HOUT), np.float32)
    for c, o in enumerate(per_core_outs):
        out[c * BPC:(c + 1) * BPC, :, :HO, :HO] = \
            np.asarray(o).astype(np.float32) + bfield[None, None]
    return out


def run(inputs, **spmd_kwargs):
    """Returns (full_output, BassKernelResults)."""
    nc = get_nc()
    in_maps = prep_inputs(**inputs)
    res = run_bass_kernel_spmd(nc, in_maps, list(range(NCORES)), **spmd_kwargs)
    return assemble([r["out"] for r in res.results], inputs["bias"]), res


def kernel(**inputs):
    out, _ = run(inputs)
    return out


# revision 14
# speedup vs baseline: 1.0232x; 1.0232x over previous
"""Trainium2 Bass kernel for a (buggy-but-well-defined) ConvTranspose2d.

Math (matches the reference exactly):
  out[b, co, i, j] = sum_{ci,kh,kw} ker[ci,co,3-kh,3-kw] * x[b,ci,i+kh-3,j+kw-3]
                     + bias_sum * cnt[i] * cnt[j]          for i,j in [0,66)
  (terms with i+kh-3 or j+kw-3 outside [0,63) are dropped), and out is zero
  elsewhere in the (B,128,126,126) output.

Strategy: data-parallel over batch (2 items / core on 8 cores).  Per core,
the 66 output rows are split into 10 groups (9x7 + 1x3 rows); each group
accumulates its [128, r*66] tile in one PSUM bank via up to 16 shifted
128x128 bf16 matmuls (contraction over ci on the partition dim).  The image
is stored UNPADDED (63x63) in SBUF: every matmul reads exactly the valid
63-wide row segments and writes a row/col-trimmed window of the PSUM tile
(out-of-range taps contribute nothing and are skipped), which cuts PE
streaming work ~9% vs the padded formulation.  start=True zeroes the whole
2KB PSUM zero-region, so partial first-tap footprints are safe.

Groups are processed in sets of {5,4,1} with the tap loop OUTER, so
consecutive matmuls share the stationary weights; a post-legalization pass
then deletes the redundant InstLdweights, which removes the per-matmul
weight-swap pipeline bubble (~45ns) for all but the first matmul of each
tap.  A burst of throwaway warm-up matmuls runs while the input DMA is in
flight so the PE's HAM clock-gate is already at 8/8 when the real stream
starts.  Everything on-chip is bf16 (fp32 PSUM accumulation); the rank-1
bias field and the zero border are applied host-side during assembly.
"""

import ml_dtypes
import numpy as np

import concourse.bacc as bacc
import concourse.mybir as mybir
import concourse.tile as tile
from concourse.bass_utils import run_bass_kernel_spmd

B, CIN, COUT, K, H, W = 16, 128, 128, 4, 64, 64
NCORES = 8
BPC = B // NCORES          # batch items per core
HV = H - 1                 # 63 valid input rows/cols
HO = HV + K - 1            # 66 output rows/cols (nonzero region)
HOUT = (H - 1) * 2         # 126 full output rows/cols
NWT = K * K * COUT         # 2048 weight cols
NXI = HV * HV              # 3969 unpadded-image cols per batch item
NXW = NWT + BPC * NXI      # merged wt+image tensor cols
F32 = mybir.dt.float32
BF16 = mybir.dt.bfloat16
BF16NP = ml_dtypes.bfloat16

GROUPS = [(0, 7), (7, 7), (14, 7), (21, 7), (28, 7),
          (35, 7), (42, 7), (49, 7), (56, 7), (63, 3)]
# Groups per set share each stationary-weight load (taps loop is outer), so
# fewer sets = fewer weight swaps.  Item 0 starts with a tiny set (the
# stream can start once ~360KB has landed); item 1 ends with a small set
# (short drain after the last matmul).  Max set size 7 = PSUM banks minus
# the warm-up bank.
SETS = {0: [(0, 1), (2, 3, 4, 5), (6, 7, 8, 9)],
        1: [(0, 1, 2, 3, 4, 5, 6), (7, 8, 9)]}
# Per-close (item, group) -> (cast engine, dma queue) overrides for the
# final closes, chosen so the two queues drain the tail in parallel.
CLOSE_OVERRIDE = {(1, 7): (0, 0), (1, 8): (1, 1), (1, 9): (0, 0)}
NWARM = 6                  # PE warm-up matmuls during the input-DMA head
NWCOL = 500                # their free dim
DEDUPE_LDW = True


def _plan(i0, r):
    """Per-group tap plan: (t, kh, kw, rs, re, ro) with zero-work taps gone."""
    plan = []
    for t in range(K * K):
        kh, kw = divmod(t, K)
        rs = max(0, i0 + kh - 3)
        re = min(HV, i0 + r + kh - 3)
        if re > rs:
            plan.append((t, kh, kw, rs, re, rs + 3 - kh - i0))
    return plan


def _dedupe_ldweights(nc):
    """Drop an InstLdweights whose weights AP matches the previous PE weight
    load with only InstMatmult/sync instructions in between: the array
    already holds those weights, and skipping the reload removes the
    per-matmul weight-swap bubble.  Only sync-free loads are dropped."""
    ndrop = 0
    for blk in nc.main_func.blocks:
        keep, prev_key = [], None
        for inst in blk.instructions:
            if getattr(inst, "engine", None) == mybir.EngineType.PE:
                if isinstance(inst, mybir.InstLdweights):
                    ap = inst.ins[0]
                    key = (str(ap.ap), ap.offset, str(ap.memref),
                           str(ap.dtype))
                    si = inst.sync_info
                    clean = si is None or (not si.on_wait and not si.on_update)
                    if key == prev_key and clean:
                        ndrop += 1
                        continue
                    prev_key = key
                elif not isinstance(inst, (mybir.InstMatmult,
                                           mybir.InstEventSemaphore)):
                    prev_key = None   # unknown PE op: don't reuse across it
            keep.append(inst)
        blk.instructions[:] = keep
    return ndrop


_CACHE = {}


def _build_nc():
    # Bacc (not raw Bass): its finalize() legalizes sync waits — moving
    # excess matmul waits onto LDWEIGHTS and splitting multi-waits onto
    # EventSemaphore instructions — which walrus codegen requires.
    nc = bacc.Bacc(None)
    xw = nc.dram_tensor("xw", [CIN, NXW], BF16, kind="ExternalInput")
    out = nc.dram_tensor("out", [BPC, COUT, HO, HO], BF16,
                         kind="ExternalOutput")

    with tile.TileContext(nc) as tc:
        with (
            tc.tile_pool(name="wpool", bufs=1) as wpool,
            tc.tile_pool(name="wps", bufs=1, space="PSUM") as wps_pool,
            tc.tile_pool(name="xwpool", bufs=1) as xwpool,
            tc.tile_pool(name="acc", bufs=7, space="PSUM") as psum_pool,
            tc.tile_pool(name="opool", bufs=6) as opool,
        ):
            # PE warm-up: dummy matmuls on a zeroed scratch tile keep the PE
            # array busy from right after the engine prologue, so the HAM
            # clock-gate reaches 8/8 before the first real matmul and the
            # input-DMA wait is hidden behind array activity.  They all share
            # one stationary load (deduped below), so the stream is dense.
            scr = wpool.tile([CIN, NWCOL], BF16)
            nc.vector.memzero(scr)
            wps = wps_pool.tile([CIN, NWCOL], F32)
            for _ in range(NWARM):
                nc.tensor.matmul(wps, scr[:, :CIN], scr, start=True, stop=True)

            xwt = xwpool.tile([CIN, NXW], BF16)
            # Input chunks in issue order.  DMA bandwidth ramps up over the
            # first ~4us, so the bytes gating the stream start are kept
            # minimal: tap kh=0 weights + the first two groups' rows
            # (~360KB).  The weights arrive per-kh row, just ahead of the
            # taps that need them.
            iw = lambda a, b: (NWT + a * HV, NWT + b * HV)  # item-0 row cols
            i1 = NWT + NXI                                  # item-1 base
            chunks = [(0, 4 * COUT), iw(0, 15),             # gate the start
                      (4 * COUT, 8 * COUT), (8 * COUT, 12 * COUT),
                      (12 * COUT, NWT),
                      iw(15, 43), iw(43, HV),
                      (i1, i1 + 50 * HV), (i1 + 50 * HV, NXW)]
            for a, b in chunks:
                nc.sync.dma_start(xwt[:, a:b], xw[:, a:b])

            xs = xwt[:, NWT:].rearrange("p (b h w) -> p b h w",
                                        b=BPC, h=HV, w=HV)

            for b in range(BPC):
                for st in SETS[b]:
                    plans = {g: _plan(*GROUPS[g]) for g in st}
                    accs = {}
                    for g in st:
                        i0, r = GROUPS[g]
                        acc = psum_pool.tile([COUT, 7 * HO], F32,
                                             tag="acc", name="acc")
                        accs[g] = acc[:, :r * HO].rearrange(
                            "p (r c) -> p r c", r=r, c=HO)
                    for t in range(K * K):
                        lhsT = xwt[:, t * COUT:(t + 1) * COUT]
                        for g in st:
                            steps = [s for s in plans[g] if s[0] == t]
                            if not steps:
                                continue
                            _, kh, kw, rs, re, ro = steps[0]
                            i0, r = GROUPS[g]
                            n = re - rs
                            nc.tensor.matmul(
                                accs[g][:, ro:ro + n, 3 - kw:HO - kw],
                                lhsT, xs[:, b, rs:re, :],
                                start=(t == plans[g][0][0]),
                                stop=(t == plans[g][-1][0]))
                            if t == plans[g][-1][0]:
                                # Close the group right after its last tap so
                                # the PSUM->SBUF cast and out-DMA overlap the
                                # remaining matmuls.  Casts alternate between
                                # DVE and ACT so bunched closes pipeline
                                # 2-wide; out-DMAs ride the Sync queue, idle
                                # after the input loads.
                                otile = opool.tile([COUT, 7 * HO], BF16,
                                                   tag="ot", name="ot")
                                flat = accs[g].rearrange("p r c -> p (r c)")
                                ce, dq = CLOSE_OVERRIDE.get((b, g),
                                                            (g % 2, g % 2))
                                if ce:
                                    nc.scalar.activation(
                                        otile[:, :r * HO], flat,
                                        mybir.ActivationFunctionType.Copy)
                                else:
                                    nc.vector.tensor_copy(
                                        otile[:, :r * HO], flat)
                                dma = nc.scalar.dma_start if dq \
                                    else nc.sync.dma_start
                                dma(out[b, :, i0:i0 + r, :],
                                    otile[:, :r * HO])
    if DEDUPE_LDW:
        _dedupe_ldweights(nc)
    nc.finalize()
    return nc


def get_nc():
    if "nc" not in _CACHE:
        _CACHE["nc"] = _build_nc()
    return _CACHE["nc"]


def prep_inputs(x, kernel, bias):
    """Host-side prep: per-core input maps (numpy only, negligible cost)."""
    x = np.asarray(x, dtype=np.float32)
    ker = np.asarray(kernel, dtype=np.float32)

    kf = ker[:, :, ::-1, ::-1]                        # [ci, co, kh, kw] flipped
    wt = np.ascontiguousarray(kf.transpose(0, 2, 3, 1)).reshape(
        CIN, NWT).astype(BF16NP)                      # [ci, (kh kw co)]
    xv = x[:, :, :HV, :HV].astype(BF16NP)             # [B, ci, 63, 63]

    in_maps = []
    for c in range(NCORES):
        xwa = np.empty((CIN, NXW), BF16NP)
        xwa[:, :NWT] = wt
        xwa[:, NWT:] = xv[c * BPC:(c + 1) * BPC].transpose(1, 0, 2, 3) \
            .reshape(CIN, BPC * NXI)
        in_maps.append({"xw": xwa})
    return in_maps


def assemble(per_core_outs, bias):
    bias = np.asarray(bias, dtype=np.float32)
    cnt = np.convolve(np.ones(HV, np.float32), np.ones(K, np.float32))
    bfield = np.sum(bias[:COUT], dtype=np.float32) * np.outer(cnt, cnt)

    out = np.zeros((B, COUT, HOUT, HOUT), np.float32)
    for c, o in enumerate(per_core_outs):
        out[c * BPC:(c + 1) * BPC, :, :HO, :HO] = \
            np.asarray(o).astype(np.float32) + bfield[None, None]
    return out


def run(inputs, **spmd_kwargs):
    """Returns (full_output, BassKernelResults)."""
    nc = get_nc()
    in_maps = prep_inputs(**inputs)
    res = run_bass_kernel_spmd(nc, in_maps, list(range(NCORES)), **spmd_kwargs)
    return assemble([r["out"] for r in res.results], inputs["bias"]), res


def kernel(**inputs):
    out, _ = run(inputs)
    return out
